# revision 12
# baseline (speedup 1.0000x reference)
"""Top-1 MoE gating + capacity-limited dispatch for Trainium2 (Bass/Tile).

Contract: kernel(**inputs) takes FULL inputs (activations [16384,3072] f32,
logits [16384,64] f32, capacity scalar int) and returns the FULL outputs
matching the reference:
    (moe_input [size,3072] f32, score [T] f32, expert_assignment [T] i32,
     mapped_slots [T] i32, expert_counts [E] i32)

Strategy (8 NeuronCores, SPMD):
  - Routing (softmax/argmax/cumsum over [T,64] logits) is small -> computed
    fully replicated on every core (no collectives needed).
  - The memory-heavy dispatch (scatter of scaled activation rows into the
    [E*cap, hidden] buffer) is sharded by OUTPUT slots: core d owns slots
    [d*2048,(d+1)*2048) == experts [8d,8d+8).  Each core builds a slot->token
    map on device and GATHERS its 2048 rows from the full activations with
    indirect DMA, scales by the gate score, and writes a contiguous
    [2048,3072] shard.  Host concatenates the 8 shards.
"""

import math
import os
import sys

import numpy as np

# Bass toolchain lives in the axon site dir (already on PYTHONPATH in-container,
# but be explicit so kernel.py is self-contained).
for _p in ("/root/.axon_site/_ro/trn_rl_repo", "/opt/trn_rl_repo"):
    if os.path.isdir(_p) and _p not in sys.path:
        sys.path.append(_p)

import concourse.bass as bass
import concourse.tile as tile
from concourse import bacc
from concourse import mybir
from concourse.masks import make_identity, make_upper_triangular

FP32 = mybir.dt.float32
I32 = mybir.dt.int32
Alu = mybir.AluOpType
Act = mybir.ActivationFunctionType

P = 128  # partitions


def build_moe_gating(T=16384, H=3072, E=64, CAP=256, N_CORES=8):
    """Build the SPMD Bass module (identical program on every core)."""
    assert T % P == 0 and E <= 64 and CAP % P == 0 and CAP <= 256
    NCH = T // P                      # token chunks
    SIZE = E * CAP                    # dispatch buffer rows (already %256==0)
    assert SIZE % 256 == 0
    SHARD = SIZE // N_CORES           # slot rows owned by each core
    NBLK = SHARD // P                 # 128-slot blocks per core
    JBLK = CAP // P                   # 128-wide j-slices per expert (1 or 2)
    assert NCH <= P                   # resident per-chunk stats tiles are [P, NCH]

    nc = bacc.Bacc(None)

    acts = nc.declare_dram_parameter("acts", [T, H], FP32, isOutput=False)
    logits = nc.declare_dram_parameter("logits", [T, E], FP32, isOutput=False)
    # per-core one-hot selector: sel[:, b] = onehot(expert of local block b),
    # stacked twice so both table halves (rows 0:E and E:2E) have an aligned rhs
    sel_in = nc.declare_dram_parameter("sel", [2 * E, NBLK], FP32, isOutput=False)

    moe_out = nc.declare_dram_parameter("moe_out", [SHARD, H], FP32, isOutput=True)
    score_out = nc.declare_dram_parameter("score_out", [NCH, P], FP32, isOutput=True)
    expert_out = nc.declare_dram_parameter("expert_out", [NCH, P], I32, isOutput=True)
    mapped_out = nc.declare_dram_parameter("mapped_out", [NCH, P], I32, isOutput=True)
    counts_out = nc.declare_dram_parameter("counts_out", [E, 1], I32, isOutput=True)

    with tile.TileContext(nc) as tc:
        with (
            tc.tile_pool(name="consts", bufs=1) as consts,
            tc.tile_pool(name="resid", bufs=1) as resid,
            tc.tile_pool(name="work", bufs=3) as work,
            tc.tile_pool(name="acttiles", bufs=4) as acttiles,
            tc.tile_pool(name="psum", bufs=4, space="PSUM") as psum,
            tc.tile_pool(name="psacc", bufs=1, space="PSUM") as psacc,
        )            :
            # ---------------- constants ----------------
            ident = consts.tile([P, P], FP32)
            make_identity(nc, ident[:])
            tri_incl = consts.tile([P, P], FP32)   # tri[k,t]=1 iff k<=t
            make_upper_triangular(nc, tri_incl[:], val=1.0, diag=True)

            iotaE_i = consts.tile([P, E], I32)     # each partition: 0..E-1
            nc.gpsimd.iota(iotaE_i[:], pattern=[[1, E]], base=0, channel_multiplier=0)
            iotaE = consts.tile([P, E], FP32)
            nc.vector.tensor_copy(iotaE[:], iotaE_i[:])
            # iota minus E (negative values) for first-argmax selection
            iotaEm = consts.tile([P, E], FP32)
            nc.vector.tensor_scalar_add(iotaEm[:], iotaE[:], float(-E))

            iotaJ_i = consts.tile([P, CAP], I32)   # each partition: 0..CAP-1
            nc.gpsimd.iota(iotaJ_i[:], pattern=[[1, CAP]], base=0, channel_multiplier=0)
            iotaJ = consts.tile([P, CAP], FP32)
            nc.vector.tensor_copy(iotaJ[:], iotaJ_i[:])

            iotaTok_i = consts.tile([P, 1], I32)   # partition index
            nc.gpsimd.iota(iotaTok_i[:], pattern=[[0, 1]], base=0, channel_multiplier=1)
            iotaTok = consts.tile([P, 1], FP32)
            nc.vector.tensor_copy(iotaTok[:], iotaTok_i[:])

            ones_col = consts.tile([P, 1], FP32)
            nc.gpsimd.memset(ones_col[:], 1.0)
            zerosEC = consts.tile([E, NCH], FP32)
            nc.gpsimd.memset(zerosEC[:], 0.0)

            sel_sb = consts.tile([2 * E, NBLK], FP32)
            nc.sync.dma_start(out=sel_sb[:], in_=sel_in[:])

            # ---------------- resident stats ----------------
            scores_all = resid.tile([P, NCH], FP32)   # 1/denom (unmasked score)
            experts_all = resid.tile([P, NCH], FP32)  # argmax expert id
            loc0_all = resid.tile([P, NCH], FP32)     # in-chunk incl. rank (1-based)
            locF_all = resid.tile([P, NCH], FP32)     # global 0-based rank
            scoreF_all = resid.tile([P, NCH], FP32)   # masked score
            mappedF_all = resid.tile([P, NCH], FP32)  # mapped_slots (or -1)
            cntT = resid.tile([E, NCH], FP32)         # per-chunk expert counts
            offT = resid.tile([E, NCH], FP32)         # exclusive prefix over chunks
            table_sb = resid.tile([P, CAP], FP32)     # rows 0:E idx, 64:64+E score

            # ============ Loop 1: per-chunk routing stats ============
            cntT_ps = psacc.tile([E, NCH], FP32, space="PSUM")
            for c in range(NCH):
                # stage through gpsimd so the ACT exp below doesn't inherit the
                # DMA queue-fanout semaphores (Activation has few wait slots)
                logit_raw = work.tile([P, E], FP32, tag="lraw")
                nc.sync.dma_start(out=logit_raw[:], in_=logits[c * P:(c + 1) * P, :])
                logit_c = work.tile([P, E], FP32, tag="logit")
                nc.gpsimd.tensor_copy(logit_c[:], logit_raw[:])

                m_c = work.tile([P, 1], FP32, tag="m")
                nc.vector.tensor_reduce(m_c[:], logit_c[:],
                                        axis=mybir.AxisListType.X, op=Alu.max)
                negm = work.tile([P, 1], FP32, tag="negm")
                nc.vector.tensor_scalar_mul(negm[:], m_c[:], -1.0)

                p_c = work.tile([P, E], FP32, tag="p")
                nc.scalar.activation(out=p_c[:], in_=logit_c[:], func=Act.Exp,
                                     bias=negm[:, 0:1], scale=1.0,
                                     accum_out=scores_all[:, c:c + 1])
                # scores_all now holds denom; reciprocal in place below
                nc.vector.reciprocal(scores_all[:, c:c + 1], scores_all[:, c:c + 1])

                # exact first-argmax: min over (e-E where logit==max else 0) + E
                eq_c = work.tile([P, E], FP32, tag="eq")
                nc.vector.tensor_tensor(out=eq_c[:], in0=logit_c[:],
                                        in1=m_c[:, 0:1].to_broadcast([P, E]),
                                        op=Alu.is_equal)
                sel_e = work.tile([P, E], FP32, tag="sele")
                nc.vector.tensor_tensor(out=sel_e[:], in0=eq_c[:], in1=iotaEm[:],
                                        op=Alu.mult)
                em_c = work.tile([P, 1], FP32, tag="em")
                nc.vector.tensor_reduce(em_c[:], sel_e[:],
                                        axis=mybir.AxisListType.X, op=Alu.min)
                nc.vector.tensor_scalar_add(experts_all[:, c:c + 1], em_c[:], float(E))

                # one-hot mask over experts
                mask_c = work.tile([P, E], FP32, tag="mask")
                nc.vector.tensor_tensor(out=mask_c[:], in0=iotaEm[:],
                                        in1=em_c[:, 0:1].to_broadcast([P, E]),
                                        op=Alu.is_equal)

                # in-chunk inclusive cumulative count: incl[t,e] = sum_{k<=t} mask[k,e]
                incl_ps = psum.tile([P, E], FP32, space="PSUM", tag="ps")
                nc.tensor.matmul(incl_ps[:], tri_incl[:], mask_c[:],
                                 start=True, stop=True)
                prod_c = work.tile([P, E], FP32, tag="prod")
                nc.vector.tensor_tensor(out=prod_c[:], in0=incl_ps[:], in1=mask_c[:],
                                        op=Alu.mult)
                nc.vector.tensor_reduce(loc0_all[:, c:c + 1], prod_c[:],
                                        axis=mybir.AxisListType.X, op=Alu.add)

                # per-chunk expert totals into transposed layout cntT[:, c]
                nc.tensor.matmul(cntT_ps[:, c:c + 1], mask_c[:], ones_col[:],
                                 start=True, stop=True)

            nc.vector.tensor_copy(cntT[:], cntT_ps[:])

            # exclusive prefix over chunks (per expert): scan along free dim
            inclT = resid.tile([E, NCH], FP32)
            nc.vector.tensor_tensor_scan(out=inclT[:], data0=cntT[:], data1=zerosEC[:],
                                         initial=0.0, op0=Alu.add, op1=Alu.add)
            nc.vector.tensor_tensor(out=offT[:], in0=inclT[:], in1=cntT[:],
                                    op=Alu.subtract)

            # expert_counts = min(total, CAP)
            cnt_min = work.tile([E, 1], FP32, tag="cntmin")
            nc.vector.tensor_scalar_min(cnt_min[:], inclT[:, NCH - 1:NCH], float(CAP))
            cnt_i32 = work.tile([E, 1], I32, tag="cnti")
            nc.vector.tensor_copy(cnt_i32[:], cnt_min[:])
            nc.sync.dma_start(out=counts_out[:], in_=cnt_i32[:])

            # ============ Loop 2a: global ranks + per-token outputs ============
            for c in range(NCH):
                mask_c = work.tile([P, E], FP32, tag="mask2")
                nc.vector.tensor_tensor(
                    out=mask_c[:], in0=iotaE[:],
                    in1=experts_all[:, c:c + 1].to_broadcast([P, E]),
                    op=Alu.is_equal)
                maskT_ps = psum.tile([E, P], FP32, space="PSUM", tag="ps")
                nc.tensor.transpose(maskT_ps[:], mask_c[:], ident[:])
                maskT_sb = work.tile([E, P], FP32, tag="maskTsb")
                nc.scalar.activation(out=maskT_sb[:], in_=maskT_ps[:], func=Act.Copy)
                off_ps = psum.tile([P, 1], FP32, space="PSUM", tag="ps")
                nc.tensor.matmul(off_ps[:], maskT_sb[:], offT[:, c:c + 1],
                                 start=True, stop=True)
                # loc = (loc0 - 1) + off
                nc.vector.scalar_tensor_tensor(
                    out=locF_all[:, c:c + 1], in0=loc0_all[:, c:c + 1], scalar=-1.0,
                    in1=off_ps[:], op0=Alu.add, op1=Alu.add)
                keep_c = work.tile([P, 1], FP32, tag="keep")
                nc.vector.tensor_scalar(keep_c[:], locF_all[:, c:c + 1],
                                        float(CAP), None, op0=Alu.is_lt)
                nc.vector.tensor_tensor(out=scoreF_all[:, c:c + 1],
                                        in0=scores_all[:, c:c + 1], in1=keep_c[:],
                                        op=Alu.mult)
                # slots = expert*CAP + loc ; mapped = (slots+1)*keep - 1
                slots_c = work.tile([P, 1], FP32, tag="slots")
                nc.vector.scalar_tensor_tensor(
                    out=slots_c[:], in0=experts_all[:, c:c + 1], scalar=float(CAP),
                    in1=locF_all[:, c:c + 1], op0=Alu.mult, op1=Alu.add)
                t1_c = work.tile([P, 1], FP32, tag="t1")
                nc.vector.scalar_tensor_tensor(
                    out=t1_c[:], in0=slots_c[:], scalar=1.0, in1=keep_c[:],
                    op0=Alu.add, op1=Alu.mult)
                nc.vector.tensor_scalar_add(mappedF_all[:, c:c + 1], t1_c[:], -1.0)

            # ============ Loop 2b: slot->token table via PE accumulation ========
            table_ps = psacc.tile([P, CAP], FP32, space="PSUM")
            for c in range(NCH):
                mask_c = work.tile([P, E], FP32, tag="mask3")
                nc.vector.tensor_tensor(
                    out=mask_c[:], in0=iotaE[:],
                    in1=experts_all[:, c:c + 1].to_broadcast([P, E]),
                    op=Alu.is_equal)
                y_c = work.tile([P, CAP], FP32, tag="y")
                nc.vector.tensor_tensor(
                    out=y_c[:], in0=iotaJ[:],
                    in1=locF_all[:, c:c + 1].to_broadcast([P, CAP]),
                    op=Alu.is_equal)
                tokid = work.tile([P, 1], FP32, tag="tokid")
                nc.vector.tensor_scalar_add(tokid[:], iotaTok[:], float(c * P))
                lhsT_c = work.tile([P, 2 * E], FP32, tag="lhsT")
                nc.vector.tensor_scalar(lhsT_c[:, 0:E], mask_c[:],
                                        tokid[:, 0:1], None, op0=Alu.mult)
                nc.vector.tensor_scalar(lhsT_c[:, E:2 * E], mask_c[:],
                                        scores_all[:, c:c + 1], None, op0=Alu.mult)
                nc.tensor.matmul(table_ps[:, :], lhsT_c[:], y_c[:],
                                 start=(c == 0), stop=(c == NCH - 1))

            nc.vector.tensor_copy(table_sb[:], table_ps[:])

            # ============ per-token outputs (transpose to token-major) ==========
            for (src, dst, as_int) in (
                (scoreF_all, score_out, False),
                (experts_all, expert_out, True),
                (mappedF_all, mapped_out, True),
            ):
                tp = psum.tile([NCH, P], FP32, space="PSUM", tag="ps")
                nc.tensor.transpose(tp[:], src[:], ident[:])
                ot = work.tile([NCH, P], I32 if as_int else FP32, tag="outsb")
                nc.vector.tensor_copy(ot[:], tp[:])
                nc.sync.dma_start(out=dst[:], in_=ot[:])

            # ============ Dispatch: gather + scale + contiguous write ==========
            for b in range(NBLK):
                jlo = (b % JBLK) * P
                idx_ps = psum.tile([P, 1], FP32, space="PSUM", tag="ps")
                nc.tensor.matmul(idx_ps[:], table_sb[0:E, jlo:jlo + P],
                                 sel_sb[0:E, b:b + 1], start=True, stop=True)
                sc_ps = psum.tile([P, 1], FP32, space="PSUM", tag="ps")
                nc.tensor.matmul(sc_ps[:], table_sb[E:2 * E, jlo:jlo + P],
                                 sel_sb[E:2 * E, b:b + 1], start=True, stop=True)
                idx_i = acttiles.tile([P, 1], I32, tag="idxi")
                nc.vector.tensor_copy(idx_i[:], idx_ps[:])
                sc_sb = acttiles.tile([P, 1], FP32, tag="scsb")
                nc.scalar.activation(out=sc_sb[:], in_=sc_ps[:], func=Act.Copy)

                act_t = acttiles.tile([P, H], FP32, tag="act")
                nc.gpsimd.indirect_dma_start(
                    out=act_t[:], out_offset=None, in_=acts[:],
                    in_offset=bass.IndirectOffsetOnAxis(ap=idx_i[:, 0:1], axis=0))
                out_t = acttiles.tile([P, H], FP32, tag="out")
                nc.vector.tensor_scalar(out_t[:], act_t[:], sc_sb[:, 0:1], None,
                                        op0=Alu.mult)
                nc.sync.dma_start(out=moe_out[b * P:(b + 1) * P, :], in_=out_t[:])

    nc.finalize()
    return nc


# ---------------------------------------------------------------------------
# host-side wrapper
# ---------------------------------------------------------------------------
_COMPILED = {}

T0, H0, E0 = 16384, 3072, 64
N_CORES = 8


def _get_nc(cap):
    key = (T0, H0, E0, cap, N_CORES)
    if key not in _COMPILED:
        _COMPILED[key] = build_moe_gating(T0, H0, E0, cap, N_CORES)
    return _COMPILED[key]


def _sel_matrix(core, cap, E=E0, n_cores=N_CORES):
    size = E * cap
    shard = size // n_cores
    nblk = shard // P
    jblk = cap // P
    sel = np.zeros((E, nblk), np.float32)
    for b in range(nblk):
        g = core * nblk + b
        sel[g // jblk, b] = 1.0
    return np.vstack([sel, sel])


def _kernel_numpy_fallback(activations, logits, capacity):
    """Pure-numpy fallback for unexpected shapes (not used for grading size)."""
    T, H = activations.shape
    E = logits.shape[1]
    x = logits - logits.max(-1, keepdims=True)
    probs = np.exp(x) / np.exp(x).sum(-1, keepdims=True)
    expert = np.argmax(logits, -1)
    mask1 = np.eye(E, dtype=np.int32)[expert]
    locations = np.cumsum(mask1, 0) - 1
    loc_s = (locations * mask1).sum(1)
    keep = loc_s < capacity
    score = probs[np.arange(T), expert]
    score = np.where(keep, score, 0.0).astype(np.float32)
    size = E * capacity
    if size % 256:
        size += 256 - size % 256
    slots = expert * capacity + loc_s
    moe = np.zeros((size, H), np.float32)
    moe[np.where(keep, slots, 0)[keep]] = (activations * score[:, None])[keep]
    mapped = np.where(keep, slots, -1).astype(np.int32)
    counts = (mask1 * keep[:, None]).sum(0).astype(np.int32)
    return moe, score, expert.astype(np.int32), mapped, counts


def kernel(activations, logits, capacity):
    from concourse.bass_utils import run_bass_kernel_spmd

    activations = np.ascontiguousarray(np.asarray(activations, np.float32))
    logits = np.ascontiguousarray(np.asarray(logits, np.float32))
    cap = int(capacity)

    if (activations.shape != (T0, H0) or logits.shape != (T0, E0)
            or cap % P != 0 or cap > 256 or (E0 * cap) % (N_CORES * 256) != 0):
        return _kernel_numpy_fallback(activations, logits, cap)

    nc = _get_nc(cap)
    in_maps = [
        {"acts": activations, "logits": logits, "sel": _sel_matrix(d, cap)}
        for d in range(N_CORES)
    ]
    res = run_bass_kernel_spmd(nc, in_maps, core_ids=list(range(N_CORES)))
    global LAST_RESULTS
    LAST_RESULTS = res
    rs = res.results
    moe = np.concatenate([rs[d]["moe_out"] for d in range(N_CORES)], axis=0)
    score = rs[0]["score_out"].reshape(-1)
    expert = rs[0]["expert_out"].reshape(-1)
    mapped = rs[0]["mapped_out"].reshape(-1)
    counts = rs[0]["counts_out"].reshape(-1)
    return moe, score, expert, mapped, counts


# revision 13
# speedup vs baseline: 1.9982x; 1.9982x over previous
"""Top-1 MoE gating + capacity-limited dispatch for Trainium2 (Bass/Tile).

Contract: kernel(**inputs) takes FULL inputs (activations [16384,3072] f32,
logits [16384,64] f32, capacity scalar int) and returns the FULL outputs
matching the reference:
    (moe_input [size,3072] f32, score [T] f32, expert_assignment [T] i32,
     mapped_slots [T] i32, expert_counts [E] i32)

Strategy (8 NeuronCores, SPMD):
  - Tokens are sharded across cores for the routing math (softmax top-1,
    argmax, in-shard ranks).  Cross-core combination needs only:
      * AllGather of per-expert shard totals S[d] (256B) -> every core gets
        its prefix base = sum_{r<d} S[r] via one tiny matmul with a per-core
        0/1 mask input (keeps all addressing compile-time in the SPMD
        program).
      * AllReduce (add) of the slot->token/score table [128, CAP] (128KB):
        each core contributes its own tokens' entries; empty entries are 0.
  - The memory-heavy dispatch is sharded by OUTPUT slots: core d owns slots
    [d*2048,(d+1)*2048) == experts [8d,8d+8).  Each core selects its slots'
    source-token indices + gate scores from the reduced table, GATHERS the
    2048 rows from the full activations with indirect DMA, scales by score,
    and writes a contiguous [2048,3072] shard.  Host concatenates shards.
"""

import os
import sys

import numpy as np

# Bass toolchain lives in the axon site dir (already on PYTHONPATH in-container,
# but be explicit so kernel.py is self-contained).
for _p in ("/root/.axon_site/_ro/trn_rl_repo", "/opt/trn_rl_repo"):
    if os.path.isdir(_p) and _p not in sys.path:
        sys.path.append(_p)

import concourse.bass as bass
import concourse.tile as tile
from concourse import bacc
from concourse import mybir
from concourse.masks import make_identity, make_upper_triangular

FP32 = mybir.dt.float32
I32 = mybir.dt.int32
Alu = mybir.AluOpType
Act = mybir.ActivationFunctionType

P = 128  # partitions


def build_moe_gating(T=16384, H=3072, E=64, CAP=256, N_CORES=8):
    """Build the SPMD Bass module (identical program on every core)."""
    assert T % (P * N_CORES) == 0 and E <= 64 and CAP % P == 0 and CAP <= 256
    NCH_G = T // P                    # global token chunks
    CH = NCH_G // N_CORES             # chunks per core
    TSH = CH * P                      # tokens per core
    SIZE = E * CAP                    # dispatch buffer rows
    assert SIZE % 256 == 0
    SHARD = SIZE // N_CORES           # slot rows owned by each core
    NBLK = SHARD // P                 # 128-slot blocks per core
    JBLK = CAP // P                   # 128-wide j-slices per expert (1 or 2)

    nc = bacc.Bacc(None)

    acts = nc.declare_dram_parameter("acts", [T, H], FP32, isOutput=False)
    # this core's token shard of the logits
    logits = nc.declare_dram_parameter("logits_sh", [TSH, E], FP32, isOutput=False)
    # per-core one-hot selector: sel[:, b] = onehot(expert of local block b),
    # stacked twice so both table halves (rows 0:E and E:2E) have an aligned rhs
    sel_in = nc.declare_dram_parameter("sel", [2 * E, NBLK], FP32, isOutput=False)
    # tok_base[p, 0] = d*TSH for all p (global id of this core's first token)
    tokb_in = nc.declare_dram_parameter("tok_base", [P, 1], FP32, isOutput=False)
    # prevm[r, 0] = 1.0 iff r < d ; prevm[r, 1] = 1.0 (for global totals)
    prevm_in = nc.declare_dram_parameter("prevm", [N_CORES, 2], FP32,
                                         isOutput=False)

    moe_out = nc.declare_dram_parameter("moe_out", [SHARD, H], FP32, isOutput=True)
    score_out = nc.declare_dram_parameter("score_out", [CH, P], FP32, isOutput=True)
    expert_out = nc.declare_dram_parameter("expert_out", [CH, P], I32, isOutput=True)
    mapped_out = nc.declare_dram_parameter("mapped_out", [CH, P], I32, isOutput=True)
    counts_out = nc.declare_dram_parameter("counts_out", [E, 1], I32, isOutput=True)

    with tile.TileContext(nc) as tc:
        with (
            tc.tile_pool(name="consts", bufs=1) as consts,
            tc.tile_pool(name="resid", bufs=1) as resid,
            tc.tile_pool(name="work", bufs=3) as work,
            tc.tile_pool(name="acttiles", bufs=4) as acttiles,
            tc.tile_pool(name="psum", bufs=4, space="PSUM") as psum,
            tc.tile_pool(name="psacc", bufs=1, space="PSUM") as psacc,
            tc.tile_pool(name="dram", bufs=1, space="DRAM") as dram,
        ):
            # ---------------- constants ----------------
            ident = consts.tile([P, P], FP32)
            make_identity(nc, ident[:])
            tri_incl = consts.tile([P, P], FP32)   # tri[k,t]=1 iff k<=t
            make_upper_triangular(nc, tri_incl[:], val=1.0, diag=True)

            iotaE_i = consts.tile([P, E], I32)     # each partition: 0..E-1
            nc.gpsimd.iota(iotaE_i[:], pattern=[[1, E]], base=0, channel_multiplier=0)
            iotaE = consts.tile([P, E], FP32)
            nc.vector.tensor_copy(iotaE[:], iotaE_i[:])
            # iota minus E (negative values) for first-argmax selection
            iotaEm = consts.tile([P, E], FP32)
            nc.vector.tensor_scalar_add(iotaEm[:], iotaE[:], float(-E))

            iotaJ_i = consts.tile([P, CAP], I32)   # each partition: 0..CAP-1
            nc.gpsimd.iota(iotaJ_i[:], pattern=[[1, CAP]], base=0, channel_multiplier=0)
            iotaJ = consts.tile([P, CAP], FP32)
            nc.vector.tensor_copy(iotaJ[:], iotaJ_i[:])

            iotaTok_i = consts.tile([P, 1], I32)   # partition index
            nc.gpsimd.iota(iotaTok_i[:], pattern=[[0, 1]], base=0, channel_multiplier=1)
            iotaTok = consts.tile([P, 1], FP32)
            nc.vector.tensor_copy(iotaTok[:], iotaTok_i[:])

            ones_col = consts.tile([P, 1], FP32)
            nc.gpsimd.memset(ones_col[:], 1.0)
            zerosEC = consts.tile([E, CH], FP32)
            nc.gpsimd.memset(zerosEC[:], 0.0)

            sel_sb = consts.tile([2 * E, NBLK], FP32)
            nc.sync.dma_start(out=sel_sb[:], in_=sel_in[:])
            tokb_sb = consts.tile([P, 1], FP32)
            nc.sync.dma_start(out=tokb_sb[:], in_=tokb_in[:])
            prevm_sb = consts.tile([N_CORES, 2], FP32)
            nc.sync.dma_start(out=prevm_sb[:], in_=prevm_in[:])

            # ---------------- resident state ----------------
            scores_all = resid.tile([P, CH], FP32)   # 1/denom (unmasked score)
            experts_all = resid.tile([P, CH], FP32)  # argmax expert id
            loc0_all = resid.tile([P, CH], FP32)     # in-chunk incl. rank (1-based)
            locF_all = resid.tile([P, CH], FP32)     # global 0-based rank
            scoreF_all = resid.tile([P, CH], FP32)   # masked score
            mappedF_all = resid.tile([P, CH], FP32)  # mapped_slots (or -1)
            cntT = resid.tile([E, CH], FP32)         # per-chunk expert counts
            offT_adj = resid.tile([E, CH], FP32)     # global exclusive prefix
            table_loc = resid.tile([P, CAP], FP32)   # this core's table part
            table_sb = resid.tile([P, CAP], FP32)    # reduced table
            Srg_sb = resid.tile([N_CORES, E], FP32)  # allgathered shard totals
            tokF = resid.tile([P, 1], FP32)          # global id of token p, chunk 0
            nc.vector.tensor_tensor(out=tokF[:], in0=iotaTok[:], in1=tokb_sb[:],
                                    op=Alu.add)

            # ============ Loop 1: per-chunk routing stats (local shard) ========
            cntT_ps = psacc.tile([E, CH], FP32, space="PSUM")
            for c in range(CH):
                # stage through gpsimd so the ACT exp below doesn't inherit the
                # DMA queue-fanout semaphores (Activation has few wait slots)
                logit_raw = work.tile([P, E], FP32, tag="lraw")
                nc.sync.dma_start(out=logit_raw[:], in_=logits[c * P:(c + 1) * P, :])
                logit_c = work.tile([P, E], FP32, tag="logit")
                nc.gpsimd.tensor_copy(logit_c[:], logit_raw[:])

                m_c = work.tile([P, 1], FP32, tag="m")
                nc.vector.tensor_reduce(m_c[:], logit_c[:],
                                        axis=mybir.AxisListType.X, op=Alu.max)
                negm = work.tile([P, 1], FP32, tag="negm")
                nc.vector.tensor_scalar_mul(negm[:], m_c[:], -1.0)

                p_c = work.tile([P, E], FP32, tag="p")
                nc.scalar.activation(out=p_c[:], in_=logit_c[:], func=Act.Exp,
                                     bias=negm[:, 0:1], scale=1.0,
                                     accum_out=scores_all[:, c:c + 1])
                # scores_all holds denom; 1/denom is the top-1 softmax score
                nc.vector.reciprocal(scores_all[:, c:c + 1], scores_all[:, c:c + 1])

                # exact first-argmax: min over (e-E where logit==max else 0) + E
                eq_c = work.tile([P, E], FP32, tag="eq")
                nc.vector.tensor_tensor(out=eq_c[:], in0=logit_c[:],
                                        in1=m_c[:, 0:1].to_broadcast([P, E]),
                                        op=Alu.is_equal)
                sel_e = work.tile([P, E], FP32, tag="sele")
                nc.vector.tensor_tensor(out=sel_e[:], in0=eq_c[:], in1=iotaEm[:],
                                        op=Alu.mult)
                em_c = work.tile([P, 1], FP32, tag="em")
                nc.vector.tensor_reduce(em_c[:], sel_e[:],
                                        axis=mybir.AxisListType.X, op=Alu.min)
                nc.vector.tensor_scalar_add(experts_all[:, c:c + 1], em_c[:], float(E))

                # one-hot mask over experts
                mask_c = work.tile([P, E], FP32, tag="mask")
                nc.vector.tensor_tensor(out=mask_c[:], in0=iotaEm[:],
                                        in1=em_c[:, 0:1].to_broadcast([P, E]),
                                        op=Alu.is_equal)

                # in-chunk inclusive cumulative count: incl[t,e] = sum_{k<=t}
                incl_ps = psum.tile([P, E], FP32, space="PSUM", tag="ps")
                nc.tensor.matmul(incl_ps[:], tri_incl[:], mask_c[:],
                                 start=True, stop=True)
                prod_c = work.tile([P, E], FP32, tag="prod")
                nc.vector.tensor_tensor(out=prod_c[:], in0=incl_ps[:], in1=mask_c[:],
                                        op=Alu.mult)
                nc.vector.tensor_reduce(loc0_all[:, c:c + 1], prod_c[:],
                                        axis=mybir.AxisListType.X, op=Alu.add)

                # per-chunk expert totals into transposed layout cntT[:, c]
                nc.tensor.matmul(cntT_ps[:, c:c + 1], mask_c[:], ones_col[:],
                                 start=True, stop=True)

            nc.vector.tensor_copy(cntT[:], cntT_ps[:])

            # shard totals S[e] -> AllGather -> prefix base + global totals
            S_loc = work.tile([E, 1], FP32, tag="Sloc")
            nc.vector.tensor_reduce(S_loc[:], cntT[:],
                                    axis=mybir.AxisListType.X, op=Alu.add)
            ag_in = dram.tile([E, 1], FP32)
            ag_out = dram.tile([N_CORES, E], FP32)
            nc.gpsimd.dma_start(out=ag_in[:], in_=S_loc[:])
            if N_CORES > 1:
                nc.gpsimd.collective_compute(
                    "AllGather", Alu.bypass,
                    replica_groups=[list(range(N_CORES))],
                    ins=[ag_in.opt()], outs=[ag_out.opt()])
            else:
                nc.gpsimd.dma_start(out=ag_out[:], in_=ag_in[:])
            nc.sync.dma_start(out=Srg_sb[:], in_=ag_out[:])

            # base[e] = sum_{r<d} S_r[e] ; tot[e] = sum_r S_r[e]
            bt_ps = psum.tile([E, 2], FP32, space="PSUM", tag="ps")
            nc.tensor.matmul(bt_ps[:], Srg_sb[:], prevm_sb[:], start=True, stop=True)
            bt_sb = work.tile([E, 2], FP32, tag="btsb")
            nc.vector.tensor_copy(bt_sb[:], bt_ps[:])

            # expert_counts = min(total, CAP)
            cnt_min = work.tile([E, 1], FP32, tag="cntmin")
            nc.vector.tensor_scalar_min(cnt_min[:], bt_sb[:, 1:2], float(CAP))
            cnt_i32 = work.tile([E, 1], I32, tag="cnti")
            nc.vector.tensor_copy(cnt_i32[:], cnt_min[:])
            nc.sync.dma_start(out=counts_out[:], in_=cnt_i32[:])

            # local exclusive prefix over chunks + global base
            inclT = resid.tile([E, CH], FP32)
            nc.vector.tensor_tensor_scan(out=inclT[:], data0=cntT[:], data1=zerosEC[:],
                                         initial=0.0, op0=Alu.add, op1=Alu.add)
            off_loc = resid.tile([E, CH], FP32)
            nc.vector.tensor_tensor(out=off_loc[:], in0=inclT[:], in1=cntT[:],
                                    op=Alu.subtract)
            nc.vector.tensor_scalar(offT_adj[:], off_loc[:], bt_sb[:, 0:1], None,
                                    op0=Alu.add)

            # ============ Loop 2a: global ranks + per-token outputs ============
            for c in range(CH):
                mask_c = work.tile([P, E], FP32, tag="mask2")
                nc.vector.tensor_tensor(
                    out=mask_c[:], in0=iotaE[:],
                    in1=experts_all[:, c:c + 1].to_broadcast([P, E]),
                    op=Alu.is_equal)
                maskT_ps = psum.tile([E, P], FP32, space="PSUM", tag="ps")
                nc.tensor.transpose(maskT_ps[:], mask_c[:], ident[:])
                maskT_sb = work.tile([E, P], FP32, tag="maskTsb")
                nc.scalar.activation(out=maskT_sb[:], in_=maskT_ps[:], func=Act.Copy)
                off_ps = psum.tile([P, 1], FP32, space="PSUM", tag="ps")
                nc.tensor.matmul(off_ps[:], maskT_sb[:], offT_adj[:, c:c + 1],
                                 start=True, stop=True)
                # loc = (loc0 - 1) + off
                nc.vector.scalar_tensor_tensor(
                    out=locF_all[:, c:c + 1], in0=loc0_all[:, c:c + 1], scalar=-1.0,
                    in1=off_ps[:], op0=Alu.add, op1=Alu.add)
                keep_c = work.tile([P, 1], FP32, tag="keep")
                nc.vector.tensor_scalar(keep_c[:], locF_all[:, c:c + 1],
                                        float(CAP), None, op0=Alu.is_lt)
                nc.vector.tensor_tensor(out=scoreF_all[:, c:c + 1],
                                        in0=scores_all[:, c:c + 1], in1=keep_c[:],
                                        op=Alu.mult)
                # slots = expert*CAP + loc ; mapped = (slots+1)*keep - 1
                slots_c = work.tile([P, 1], FP32, tag="slots")
                nc.vector.scalar_tensor_tensor(
                    out=slots_c[:], in0=experts_all[:, c:c + 1], scalar=float(CAP),
                    in1=locF_all[:, c:c + 1], op0=Alu.mult, op1=Alu.add)
                t1_c = work.tile([P, 1], FP32, tag="t1")
                nc.vector.scalar_tensor_tensor(
                    out=t1_c[:], in0=slots_c[:], scalar=1.0, in1=keep_c[:],
                    op0=Alu.add, op1=Alu.mult)
                nc.vector.tensor_scalar_add(mappedF_all[:, c:c + 1], t1_c[:], -1.0)

            # ============ Loop 2b: local slot->token table via PE accum ========
            table_ps = psacc.tile([P, CAP], FP32, space="PSUM")
            for c in range(CH):
                mask_c = work.tile([P, E], FP32, tag="mask3")
                nc.vector.tensor_tensor(
                    out=mask_c[:], in0=iotaE[:],
                    in1=experts_all[:, c:c + 1].to_broadcast([P, E]),
                    op=Alu.is_equal)
                y_c = work.tile([P, CAP], FP32, tag="y")
                nc.vector.tensor_tensor(
                    out=y_c[:], in0=iotaJ[:],
                    in1=locF_all[:, c:c + 1].to_broadcast([P, CAP]),
                    op=Alu.is_equal)
                tokid = work.tile([P, 1], FP32, tag="tokid")
                nc.vector.tensor_scalar_add(tokid[:], tokF[:], float(c * P))
                lhsT_c = work.tile([P, 2 * E], FP32, tag="lhsT")
                nc.vector.tensor_scalar(lhsT_c[:, 0:E], mask_c[:],
                                        tokid[:, 0:1], None, op0=Alu.mult)
                nc.vector.tensor_scalar(lhsT_c[:, E:2 * E], mask_c[:],
                                        scores_all[:, c:c + 1], None, op0=Alu.mult)
                nc.tensor.matmul(table_ps[:, :], lhsT_c[:], y_c[:],
                                 start=(c == 0), stop=(c == CH - 1))

            nc.vector.tensor_copy(table_loc[:], table_ps[:])

            # AllReduce the table (each core contributed its own tokens)
            ar_in = dram.tile([P, CAP], FP32)
            ar_out = dram.tile([P, CAP], FP32)
            nc.gpsimd.dma_start(out=ar_in[:], in_=table_loc[:])
            if N_CORES > 1:
                nc.gpsimd.collective_compute(
                    "AllReduce", Alu.add,
                    replica_groups=[list(range(N_CORES))],
                    ins=[ar_in.opt()], outs=[ar_out.opt()])
            else:
                nc.gpsimd.dma_start(out=ar_out[:], in_=ar_in[:])
            nc.sync.dma_start(out=table_sb[:], in_=ar_out[:])

            # ============ per-token outputs (transpose to token-major) ==========
            for (src, dst, as_int) in (
                (scoreF_all, score_out, False),
                (experts_all, expert_out, True),
                (mappedF_all, mapped_out, True),
            ):
                tp = psum.tile([CH, P], FP32, space="PSUM", tag="ps")
                nc.tensor.transpose(tp[:], src[:], ident[:])
                ot = work.tile([CH, P], I32 if as_int else FP32, tag="outsb")
                nc.vector.tensor_copy(ot[:], tp[:])
                nc.sync.dma_start(out=dst[:], in_=ot[:])

            # ============ Dispatch: gather + scale + contiguous write ==========
            for b in range(NBLK):
                jlo = (b % JBLK) * P
                idx_ps = psum.tile([P, 1], FP32, space="PSUM", tag="ps")
                nc.tensor.matmul(idx_ps[:], table_sb[0:E, jlo:jlo + P],
                                 sel_sb[0:E, b:b + 1], start=True, stop=True)
                sc_ps = psum.tile([P, 1], FP32, space="PSUM", tag="ps")
                nc.tensor.matmul(sc_ps[:], table_sb[E:2 * E, jlo:jlo + P],
                                 sel_sb[E:2 * E, b:b + 1], start=True, stop=True)
                idx_i = acttiles.tile([P, 1], I32, tag="idxi")
                nc.vector.tensor_copy(idx_i[:], idx_ps[:])
                sc_sb = acttiles.tile([P, 1], FP32, tag="scsb")
                nc.scalar.activation(out=sc_sb[:], in_=sc_ps[:], func=Act.Copy)

                act_t = acttiles.tile([P, H], FP32, tag="act")
                nc.gpsimd.indirect_dma_start(
                    out=act_t[:], out_offset=None, in_=acts[:],
                    in_offset=bass.IndirectOffsetOnAxis(ap=idx_i[:, 0:1], axis=0))
                out_t = acttiles.tile([P, H], FP32, tag="out")
                nc.vector.tensor_scalar(out_t[:], act_t[:], sc_sb[:, 0:1], None,
                                        op0=Alu.mult)
                nc.sync.dma_start(out=moe_out[b * P:(b + 1) * P, :], in_=out_t[:])

    nc.finalize()
    return nc


# ---------------------------------------------------------------------------
# host-side wrapper
# ---------------------------------------------------------------------------
_COMPILED = {}
LAST_RESULTS = None

T0, H0, E0 = 16384, 3072, 64
N_CORES = 8


def _get_nc(cap):
    key = (T0, H0, E0, cap, N_CORES)
    if key not in _COMPILED:
        _COMPILED[key] = build_moe_gating(T0, H0, E0, cap, N_CORES)
    return _COMPILED[key]


def _sel_matrix(core, cap, E=E0, n_cores=N_CORES):
    size = E * cap
    shard = size // n_cores
    nblk = shard // P
    jblk = cap // P
    sel = np.zeros((E, nblk), np.float32)
    for b in range(nblk):
        g = core * nblk + b
        sel[g // jblk, b] = 1.0
    return np.vstack([sel, sel])


def _core_inputs(core, activations, logits, cap, n_cores=N_CORES):
    T, _H = activations.shape
    E = logits.shape[1]
    tsh = T // n_cores
    prevm = np.zeros((n_cores, 2), np.float32)
    prevm[:core, 0] = 1.0
    prevm[:, 1] = 1.0
    return {
        "acts": activations,
        "logits_sh": np.ascontiguousarray(logits[core * tsh:(core + 1) * tsh]),
        "sel": _sel_matrix(core, cap, E, n_cores),
        "tok_base": np.full((P, 1), core * tsh, np.float32),
        "prevm": prevm,
    }


def _kernel_numpy_fallback(activations, logits, capacity):
    """Pure-numpy fallback for unexpected shapes (not used for grading size)."""
    T, H = activations.shape
    E = logits.shape[1]
    x = logits - logits.max(-1, keepdims=True)
    probs = np.exp(x) / np.exp(x).sum(-1, keepdims=True)
    expert = np.argmax(logits, -1)
    mask1 = np.eye(E, dtype=np.int32)[expert]
    locations = np.cumsum(mask1, 0) - 1
    loc_s = (locations * mask1).sum(1)
    keep = loc_s < capacity
    score = probs[np.arange(T), expert]
    score = np.where(keep, score, 0.0).astype(np.float32)
    size = E * capacity
    if size % 256:
        size += 256 - size % 256
    slots = expert * capacity + loc_s
    moe = np.zeros((size, H), np.float32)
    moe[np.where(keep, slots, 0)[keep]] = (activations * score[:, None])[keep]
    mapped = np.where(keep, slots, -1).astype(np.int32)
    counts = (mask1 * keep[:, None]).sum(0).astype(np.int32)
    return moe, score, expert.astype(np.int32), mapped, counts


def kernel(activations, logits, capacity):
    from concourse.bass_utils import run_bass_kernel_spmd

    activations = np.ascontiguousarray(np.asarray(activations, np.float32))
    logits = np.ascontiguousarray(np.asarray(logits, np.float32))
    cap = int(capacity)

    if (activations.shape != (T0, H0) or logits.shape != (T0, E0)
            or cap % P != 0 or cap > 256 or (E0 * cap) % (N_CORES * 256) != 0):
        return _kernel_numpy_fallback(activations, logits, cap)

    nc = _get_nc(cap)
    in_maps = [_core_inputs(d, activations, logits, cap) for d in range(N_CORES)]
    res = run_bass_kernel_spmd(nc, in_maps, core_ids=list(range(N_CORES)))
    global LAST_RESULTS
    LAST_RESULTS = res
    rs = res.results
    moe = np.concatenate([rs[d]["moe_out"] for d in range(N_CORES)], axis=0)
    score = np.concatenate([rs[d]["score_out"].reshape(-1) for d in range(N_CORES)])
    expert = np.concatenate([rs[d]["expert_out"].reshape(-1) for d in range(N_CORES)])
    mapped = np.concatenate([rs[d]["mapped_out"].reshape(-1) for d in range(N_CORES)])
    counts = rs[0]["counts_out"].reshape(-1)
    return moe, score, expert, mapped, counts
